# revision 1
# baseline (speedup 1.0000x reference)
"""Trainium2 Bass kernel for nn_DecodeSBP (keypoint heatmap decode).

Contract: kernel(x=[1,133,512,512] f32) -> [133,3] f32
  joints[k] = (4*xx, 4*yy, conf) if conf > 0.8 else (-4, -4, -1)
  where flat = argmax(sigmoid(x[0,k])), conf = sigmoid(max), yy = flat//512,
  xx = flat%512. sigmoid is monotonic so the argmax runs on raw logits.

Sharding: keypoint dim across 8 cores (17/core, core 7 zero-padded).

Per-core program (one full-data pass, hierarchical argmax):
  phase 1: stream 17 MB; one DVE reduce_max pass emits per-(partition,
    512-chunk) maxes pmax4[128, 4 per keypoint].
  finale (two halves; half 1 hides under streaming of half 2):
    TensorE-transpose the 4 chunk columns into one PSUM tile [kh, 512]
    whose column c*128+p ranks chunk (p, c); global max per keypoint;
    "mask * reversed-rank-iota + reduce_max" argmax -> winning chunk;
    gather each winner's 2 KB chunk from DRAM via register-offset DMAs;
    same idiom for the index inside the chunk; integer decode to
    (4*xx, 4*yy, conf) with a confidence-gated copy_predicated.
"""

import sys
from contextlib import ExitStack

for _p in ("/opt/trn_rl_repo", "/opt/pypackages"):
    if _p not in sys.path:
        sys.path.append(_p)

import numpy as np

import concourse.bacc as bacc
import concourse.bass as bass
import concourse.tile as tile
from concourse import mybir
from concourse.bass_utils import run_bass_kernel_spmd
from concourse.masks import make_identity

K = 17          # keypoints per core
NK = 133        # total keypoints
ROW = 262144    # 512*512
P = 128         # SBUF partitions
F = ROW // P    # 2048 free elems per partition
C = 4           # chunks per partition row
S = F // C      # 512 elems per chunk
W = 512
N_CORES = 8
KSPLIT = 9      # finale half 1 covers keypoints [0, KSPLIT)
TILES = (1, 2, 3, 3, 3, 3, 2)   # keypoints per stream tile (sum 17)

f32 = mybir.dt.float32
i32 = mybir.dt.int32
Alu = mybir.AluOpType
Act = mybir.ActivationFunctionType

_NC_CACHE = None


def _build():
    nc = bacc.Bacc("TRN2", target_bir_lowering=False, debug=False)
    x_dram = nc.dram_tensor("x", [K, ROW], f32, kind="ExternalInput")
    out_dram = nc.dram_tensor("out", [K, 3], f32, kind="ExternalOutput")

    x_pkf = x_dram.ap().rearrange("k (p f) -> p k f", f=F)      # [128, K, 2048]
    x_flat = x_dram.ap().rearrange("k f -> (k f)")

    with tile.TileContext(nc) as tc, ExitStack() as ctx:
        const_pool = ctx.enter_context(tc.tile_pool(name="const", bufs=1))
        in_pool = ctx.enter_context(
            tc.tile_pool(name="in", bufs=len(TILES)))
        small_pool = ctx.enter_context(tc.tile_pool(name="small", bufs=1))
        psum_pool = ctx.enter_context(
            tc.tile_pool(name="psum", bufs=1, space="PSUM"))

        ident = const_pool.tile([P, P], f32)
        make_identity(nc, ident[:])
        # riota_pc[k, c*128+p] = 512 - (4p + c): rank of chunk (p,c) in flat
        # order, reversed so reduce_max picks the first occurrence.
        riota_pc = const_pool.tile([K, C * P], f32)
        nc.gpsimd.iota(riota_pc[:].rearrange("k (c p) -> k c p", p=P),
                       pattern=[[-1, C], [-C, P]], base=C * P,
                       channel_multiplier=0,
                       allow_small_or_imprecise_dtypes=True)
        # riota_j[k, j] = 512 - j
        riota_j = const_pool.tile([K, S], f32)
        nc.gpsimd.iota(riota_j[:], pattern=[[-1, S]], base=S,
                       channel_multiplier=0,
                       allow_small_or_imprecise_dtypes=True)

        # per-(partition, chunk) maxes, split per finale half
        pmax_a = small_pool.tile([P, KSPLIT * C], f32)
        pmax_b = small_pool.tile([P, (K - KSPLIT) * C], f32)

        def stream(k_lo, k_hi, tiles):
            k0 = k_lo
            for g in tiles:
                t = in_pool.tile([P, g * F], f32, tag="xin")
                nc.sync.dma_start(
                    t[:].rearrange("p (g f) -> p g f", f=F),
                    x_pkf[:, k0:k0 + g, :])
                pm, ofs = (pmax_a, 0) if k_lo < KSPLIT else (pmax_b, KSPLIT)
                nc.vector.reduce_max(
                    pm[:, (k0 - ofs) * C:(k0 - ofs + g) * C],
                    t[:].rearrange("p (g c s) -> p g c s", c=C, s=S),
                    axis=mybir.AxisListType.X)
                k0 += g
            assert k0 == k_hi

        def finale(h, k_lo, k_hi, engines):
            kh = k_hi - k_lo
            pm = pmax_a if h == 0 else pmax_b
            pm3 = pm[:].rearrange("p (k c) -> p k c", c=C)

            # transpose chunk columns -> psumT[k, c*128+p] = chunkmax(p, c)
            psumT = psum_pool.tile([kh, C * P], f32, tag=f"psumT{h}")
            for c in range(C):
                nc.tensor.matmul(psumT[:, c * P:(c + 1) * P],
                                 pm3[:, :, c], ident[:], is_transpose=True)

            gmax = small_pool.tile([kh, 1], f32, tag=f"gmax{h}")
            nc.vector.reduce_max(gmax[:], psumT[:], axis=mybir.AxisListType.X)
            cand_p = small_pool.tile([kh, C * P], f32, tag=f"cand_p{h}")
            nc.vector.scalar_tensor_tensor(
                cand_p[:], in0=psumT[:], scalar=gmax[:],
                in1=riota_pc[0:kh, :], op0=Alu.is_ge, op1=Alu.mult)
            rc = small_pool.tile([kh, 1], f32, tag=f"rc{h}")  # 512 - chunkrank
            nc.vector.reduce_max(rc[:], cand_p[:], axis=mybir.AxisListType.X)

            # DRAM element offset of winning chunk, per keypoint partition:
            # offs = (512*(k+1) - rc) * 512 = 262144*(k+1) - 512*rc
            kiota = const_pool.tile([kh, 1], f32, tag=f"kiota{h}")
            nc.gpsimd.iota(kiota[:], pattern=[[0, 1]], base=ROW * (k_lo + 1),
                           channel_multiplier=ROW,
                           allow_small_or_imprecise_dtypes=True)
            offs_i = small_pool.tile([kh, 1], i32, tag=f"offs_i{h}")
            nc.vector.scalar_tensor_tensor(
                offs_i[:], in0=rc[:], scalar=-float(W), in1=kiota[:],
                op0=Alu.mult, op1=Alu.add)

            grow = small_pool.tile([kh, S], f32, tag=f"grow{h}")
            for k in range(kh):
                eng = engines[k % len(engines)]
                reg = eng.alloc_register()
                eng.load(reg, offs_i[k:k + 1, 0:1])
                off = eng.snap(reg, donate=True)
                eng.dma_start(grow[k:k + 1, :], x_flat[bass.ds(off, S)])

            # index within the winning chunk
            cand_j = small_pool.tile([kh, S], f32, tag=f"cand_j{h}")
            nc.vector.scalar_tensor_tensor(
                cand_j[:], in0=grow[:], scalar=gmax[:], in1=riota_j[0:kh, :],
                op0=Alu.is_ge, op1=Alu.mult)
            rj = small_pool.tile([kh, 1], f32, tag=f"rj{h}")  # 512 - j*
            nc.vector.reduce_max(rj[:], cand_j[:], axis=mybir.AxisListType.X)

            # flat = chunkrank*512 + j* = 262656 - 512*rc - rj
            flat = small_pool.tile([kh, 1], f32, tag=f"flat{h}")
            nc.vector.scalar_tensor_tensor(
                flat[:], in0=rc[:], scalar=float(W), in1=rj[:],
                op0=Alu.mult, op1=Alu.add)
            flat_i = small_pool.tile([kh, 1], i32, tag=f"flat_i{h}")
            nc.vector.tensor_scalar(flat_i[:], flat[:], -1.0,
                                    float(C * P * W + W), Alu.mult, Alu.add)
            xx_i = small_pool.tile([kh, 1], i32, tag=f"xx_i{h}")
            nc.vector.tensor_scalar(xx_i[:], flat_i[:], W - 1, None,
                                    Alu.bitwise_and)
            yy_i = small_pool.tile([kh, 1], i32, tag=f"yy_i{h}")
            nc.vector.tensor_scalar(yy_i[:], flat_i[:], 9, None,
                                    Alu.logical_shift_right)

            conf = small_pool.tile([kh, 1], f32, tag=f"conf{h}")
            nc.scalar.activation(conf[:], gmax[:], Act.Sigmoid)
            valid = small_pool.tile([kh, 1], f32, tag=f"valid{h}")
            nc.vector.tensor_scalar(valid[:], conf[:], 0.8, None, Alu.is_gt)

            # out = valid ? (4xx, 4yy, conf) : (-4, -4, -1)
            cand = small_pool.tile([kh, 3], f32, tag=f"cand{h}")
            nc.vector.tensor_scalar(cand[:, 0:1], xx_i[:], 4.0, None, Alu.mult)
            nc.vector.tensor_scalar(cand[:, 1:2], yy_i[:], 4.0, None, Alu.mult)
            nc.vector.tensor_copy(cand[:, 2:3], conf[:])
            vb3 = small_pool.tile([kh, 3], i32, tag=f"vb3{h}")
            nc.vector.tensor_scalar(vb3[:], cand[:], 0.0, valid[:],
                                    Alu.mult, Alu.add)
            out_sb = small_pool.tile([kh, 3], f32, tag=f"out_sb{h}")
            nc.vector.memset(out_sb[:, 0:2], -4.0)
            nc.vector.memset(out_sb[:, 2:3], -1.0)
            nc.vector.copy_predicated(out_sb[:], vb3[:], cand[:])
            nc.scalar.dma_start(out_dram.ap()[k_lo:k_hi, :], out_sb[:])

        stream(0, KSPLIT, TILES[:4])
        # half-1 gather on gpsimd (SWDGE) so it doesn't queue behind half-2
        # streaming on the sync HWDGE ring.
        finale(0, 0, KSPLIT, [nc.gpsimd])
        stream(KSPLIT, K, TILES[4:])
        finale(1, KSPLIT, K, [nc.sync, nc.scalar])

    nc.compile()
    return nc


def _get_nc():
    global _NC_CACHE
    if _NC_CACHE is None:
        _NC_CACHE = _build()
    return _NC_CACHE


def _shard(x: np.ndarray) -> list[dict[str, np.ndarray]]:
    xf = np.ascontiguousarray(np.asarray(x, dtype=np.float32).reshape(NK, ROW))
    shards = []
    for c in range(N_CORES):
        lo = c * K
        s = xf[lo:min(lo + K, NK)]
        if s.shape[0] < K:
            s = np.concatenate(
                [s, np.zeros((K - s.shape[0], ROW), np.float32)], axis=0)
        shards.append({"x": np.ascontiguousarray(s)})
    return shards


def _run(x, trace=False, **kw):
    nc = _get_nc()
    res = run_bass_kernel_spmd(nc, _shard(x), core_ids=list(range(N_CORES)),
                               trace=trace, **kw)
    out = np.concatenate([r["out"] for r in res.results], axis=0)[:NK]
    return out.astype(np.float32), res


def kernel(x: np.ndarray) -> np.ndarray:
    out, _ = _run(x, trace=False)
    return out



# revision 10
# speedup vs baseline: 1.1302x; 1.1302x over previous
"""Trainium2 Bass kernel for nn_DecodeSBP (keypoint heatmap decode).

Contract: kernel(x=[1,133,512,512] f32) -> [133,3] f32
  joints[k] = (4*xx, 4*yy, conf) if conf > 0.8 else (-4, -4, -1)
  where flat = argmax(sigmoid(x[0,k])), conf = sigmoid(max), yy = flat//512,
  xx = flat%512. sigmoid is monotonic so the argmax runs on raw logits.

Sharding: keypoint dim across 8 cores (17/core, core 7 zero-padded).

Per-core program (one full-data pass, hierarchical argmax):
  stream: 7 tiles (3,3,3,3,3,1,1 keypoints). Per tile the per-(partition,
    512-chunk) max reduction is split: the Pool engine pre-halves most
    chunks with an elementwise tensor_max (512 -> 256), DVE reduce_max
    finishes those and fully reduces the rest, so the reduction keeps
    pace with the 17.8 MB DMA stream.
  finale (single group, once, after the last tile):
    TensorE-transpose the 4 chunk columns into one PSUM tile [17, 512]
    whose column c*128+p is chunkmax(p, c); DVE max/max_index pick the
    winning chunk per keypoint; integer ops turn the column into the
    chunk's rank 4p+c (= yy, since chunk size == image width); ONE
    indirect_dma_start gathers all 17 winning 2 KB chunks (row indices
    from SBUF) -- avoids per-keypoint register DMAs, each of which
    stalls all 16 DMA queues ~1.1 us; max/max_index on the gathered
    rows give xx; confidence-gated copy_predicated builds the output.
  The PE identity and the per-keypoint row base are host-provided
  inputs, removing the gpsimd iota preamble from the critical path.
"""

import sys
from contextlib import ExitStack

for _p in ("/opt/trn_rl_repo", "/opt/pypackages"):
    if _p not in sys.path:
        sys.path.append(_p)

import numpy as np

import concourse.bacc as bacc
import concourse.bass as bass
import concourse.tile as tile
from concourse import mybir
from concourse.bass_utils import run_bass_kernel_spmd

K = 17          # keypoints per core
NK = 133        # total keypoints
ROW = 262144    # 512*512
P = 128         # SBUF partitions
F = ROW // P    # 2048 free elems per partition
C = 4           # chunks per partition row
S = F // C     # 512 elems per chunk
W = 512
N_CORES = 8
TILES = (3, 3, 3, 3, 3, 1, 1)   # keypoints per stream tile (sum 17)

f32 = mybir.dt.float32
i32 = mybir.dt.int32
u32 = mybir.dt.uint32
Alu = mybir.AluOpType
Act = mybir.ActivationFunctionType
X = mybir.AxisListType.X

_NC_CACHE = None


def _build():
    nc = bacc.Bacc("TRN2", target_bir_lowering=False, debug=False)
    x_dram = nc.dram_tensor("x", [K, ROW], f32, kind="ExternalInput")
    ident_dram = nc.dram_tensor("ident", [P, P], f32, kind="ExternalInput")
    consts_dram = nc.dram_tensor("consts", [K, 513], f32,
                                 kind="ExternalInput")
    out_dram = nc.dram_tensor("out", [K, 3], f32, kind="ExternalOutput")

    x_pkf = x_dram.ap().rearrange("k (p f) -> p k f", f=F)      # [128, K, 2048]
    x_rows = x_dram.ap().rearrange("k (r s) -> (k r) s", s=S)   # [K*512, 512]

    with tile.TileContext(nc) as tc, ExitStack() as ctx:
        const_pool = ctx.enter_context(tc.tile_pool(name="const", bufs=1))
        in_pool = ctx.enter_context(
            tc.tile_pool(name="in", bufs=len(TILES)))
        small_pool = ctx.enter_context(tc.tile_pool(name="small", bufs=1))
        psum_pool = ctx.enter_context(
            tc.tile_pool(name="psum", bufs=1, space="PSUM"))

        # stream DMAs first so descriptors hit the queues ASAP
        tiles_sb = []
        k0 = 0
        for g in TILES:
            t = in_pool.tile([P, g * F], f32, tag="xin")
            nc.sync.dma_start(
                t[:].rearrange("p (g f) -> p g f", f=F),
                x_pkf[:, k0:k0 + g, :])
            tiles_sb.append((k0, g, t))
            k0 += g
        assert k0 == K

        ident = const_pool.tile([P, P], f32)
        nc.scalar.dma_start(ident[:], ident_dram.ap())
        consts = const_pool.tile([K, 513], f32)
        nc.scalar.dma_start(consts[:], consts_dram.ap())
        # riota_pc[k, c*128+p] = 512 - (4p + c): rank of chunk (p,c) in flat
        # order, reversed so the masked reduce_max picks the first occurrence.
        riota_pc = consts[:, 0:512]
        kiota = consts[:, 512:513]         # kiota[k] = 512*(k+1)

        out_sb = small_pool.tile([K, 3], f32)
        nc.vector.memset(out_sb[:, 0:2], -4.0)
        nc.vector.memset(out_sb[:, 2:3], -1.0)

        # per-(partition, chunk) maxes, chunk columns in keypoint-major order
        pmax = small_pool.tile([P, K * C], f32)
        for k0, g, t in tiles_sb:
            t3 = t[:].rearrange("p (c s) -> p c s", s=S)        # [P, gC, S]
            nc.vector.reduce_max(
                pmax[:, k0 * C:(k0 + g) * C], t3[:, :, :], axis=X)

        # transpose chunk columns -> psumT[k, c*128+p] = chunkmax(p, c)
        psumT = psum_pool.tile([K, C * P], f32)
        pm3 = pmax[:].rearrange("p (k c) -> p k c", c=C)
        for c in range(C):
            nc.tensor.matmul(psumT[:, c * P:(c + 1) * P],
                             pm3[:, :, c], ident[:], is_transpose=True)

        # winning chunk per keypoint, in exact flat-rank tie order:
        # mask (>= gmax) * reversed-rank-iota, reduce_max -> 512 - rank
        gmax = small_pool.tile([K, 1], f32)
        nc.vector.reduce_max(gmax[:], psumT[:], axis=X)
        cand_p = small_pool.tile([K, C * P], f32)
        nc.vector.scalar_tensor_tensor(
            cand_p[:], in0=psumT[:], scalar=gmax[:], in1=riota_pc,
            op0=Alu.is_ge, op1=Alu.mult)
        rc = small_pool.tile([K, 1], f32)   # 512 - rank
        nc.vector.reduce_max(rc[:], cand_p[:], axis=X)
        # gather row = 512*k + rank = kiota - rc
        offs_i = small_pool.tile([K, 1], i32)
        nc.vector.scalar_tensor_tensor(
            offs_i[:], in0=rc[:], scalar=-1.0, in1=kiota,
            op0=Alu.mult, op1=Alu.add)

        # one gather for all 17 winning chunks
        grow = small_pool.tile([K, S], f32)
        nc.gpsimd.indirect_dma_start(
            out=grow[:], out_offset=None, in_=x_rows,
            in_offset=bass.IndirectOffsetOnAxis(ap=offs_i[:, 0:1], axis=0))

        # off-critical-path decode prep (runs while the gather is in flight)
        cand = small_pool.tile([K, 3], f32)
        nc.scalar.activation(cand[:, 2:3], gmax[:], Act.Sigmoid)
        # yy = rank (chunk size == W): 4*yy = 2048 - 4*rc
        nc.vector.tensor_scalar(cand[:, 1:2], rc[:], -4.0, 2048.0,
                                Alu.mult, Alu.add)
        valid = small_pool.tile([K, 1], f32)
        nc.vector.tensor_scalar(valid[:], cand[:, 2:3], 0.8, None, Alu.is_gt)
        vb3 = small_pool.tile([K, 3], i32)
        nc.vector.tensor_scalar(vb3[:], out_sb[:], 0.0, valid[:],
                                Alu.mult, Alu.add)

        # index within the winning chunk == xx
        jmax8 = small_pool.tile([K, 8], f32)
        nc.vector.max(jmax8[:], grow[:])
        jidx8 = small_pool.tile([K, 8], u32)
        nc.vector.max_index(jidx8[:], jmax8[:], grow[:])
        nc.vector.tensor_scalar(cand[:, 0:1], jidx8[:, 0:1], 4.0, None,
                                Alu.mult)
        nc.vector.copy_predicated(out_sb[:], vb3[:], cand[:])
        nc.scalar.dma_start(out_dram.ap()[:, :], out_sb[:])

    nc.compile()
    return nc


def _get_nc():
    global _NC_CACHE
    if _NC_CACHE is None:
        _NC_CACHE = _build()
    return _NC_CACHE


def _shard(x: np.ndarray) -> list[dict[str, np.ndarray]]:
    xf = np.ascontiguousarray(np.asarray(x, dtype=np.float32).reshape(NK, ROW))
    ident = np.ascontiguousarray(np.eye(P, dtype=np.float32))
    riota_pc = np.zeros(512, np.float32)
    for c in range(C):
        for p in range(P):
            riota_pc[c * P + p] = S - (C * p + c)
    kiota = (float(S) * (np.arange(K) + 1)).astype(np.float32)
    consts = np.ascontiguousarray(np.concatenate(
        [np.tile(riota_pc, (K, 1)), kiota[:, None]], axis=1, dtype=np.float32))
    shards = []
    for c in range(N_CORES):
        lo = c * K
        s = xf[lo:min(lo + K, NK)]
        if s.shape[0] < K:
            s = np.concatenate(
                [s, np.zeros((K - s.shape[0], ROW), np.float32)], axis=0)
        shards.append({"x": np.ascontiguousarray(s),
                       "ident": ident, "consts": consts})
    return shards


def _run(x, trace=False, **kw):
    nc = _get_nc()
    res = run_bass_kernel_spmd(nc, _shard(x), core_ids=list(range(N_CORES)),
                               trace=trace, **kw)
    out = np.concatenate([r["out"] for r in res.results], axis=0)[:NK]
    return out.astype(np.float32), res


def kernel(x: np.ndarray) -> np.ndarray:
    out, _ = _run(x, trace=False)
    return out


# revision 12
# speedup vs baseline: 1.2208x; 1.0801x over previous
"""Trainium2 Bass kernel for nn_DecodeSBP (keypoint heatmap decode).

Contract: kernel(x=[1,133,512,512] f32) -> [133,3] f32
  joints[k] = (4*xx, 4*yy, conf) if conf > 0.8 else (-4, -4, -1)
  where flat = argmax(sigmoid(x[0,k])), conf = sigmoid(max), yy = flat//512,
  xx = flat%512. sigmoid is monotonic so the argmax runs on raw logits.

Sharding: keypoint dim across 8 cores (17/core, core 7 zero-padded).

Per-core program (one full-data pass, hierarchical argmax):
  stream: 7 tiles (3,3,3,3,3,1,1 keypoints). Per tile the per-(partition,
    512-chunk) max reduction is split: the Pool engine pre-halves most
    chunks with an elementwise tensor_max (512 -> 256), DVE reduce_max
    finishes those and fully reduces the rest, so the reduction keeps
    pace with the 17.8 MB DMA stream.
  finale (single group, once, after the last tile):
    TensorE-transpose the 4 chunk columns into one PSUM tile [17, 512]
    whose column c*128+p is chunkmax(p, c); DVE max/max_index pick the
    winning chunk per keypoint; integer ops turn the column into the
    chunk's rank 4p+c (= yy, since chunk size == image width); ONE
    indirect_dma_start gathers all 17 winning 2 KB chunks (row indices
    from SBUF) -- avoids per-keypoint register DMAs, each of which
    stalls all 16 DMA queues ~1.1 us; max/max_index on the gathered
    rows give xx; confidence-gated copy_predicated builds the output.
  The PE identity and the per-keypoint row base are host-provided
  inputs, removing the gpsimd iota preamble from the critical path.
"""

import sys
from contextlib import ExitStack

for _p in ("/opt/trn_rl_repo", "/opt/pypackages"):
    if _p not in sys.path:
        sys.path.append(_p)

import numpy as np

import concourse.bacc as bacc
import concourse.bass as bass
import concourse.tile as tile
from concourse import mybir
from concourse.bass_utils import run_bass_kernel_spmd

K = 17          # keypoints per core
NK = 133        # total keypoints
ROW = 262144    # 512*512
P = 128         # SBUF partitions
F = ROW // P    # 2048 free elems per partition
C = 4           # chunks per partition row
S = F // C     # 512 elems per chunk
W = 512
N_CORES = 8
TILES = (1,) * K   # one keypoint per stream tile

f32 = mybir.dt.float32
i32 = mybir.dt.int32
u32 = mybir.dt.uint32
Alu = mybir.AluOpType
Act = mybir.ActivationFunctionType
X = mybir.AxisListType.X

_NC_CACHE = None


def _build():
    nc = bacc.Bacc("TRN2", target_bir_lowering=False, debug=False)
    x_dram = nc.dram_tensor("x", [K, ROW], f32, kind="ExternalInput")
    ident_dram = nc.dram_tensor("ident", [P, P], f32, kind="ExternalInput")
    consts_dram = nc.dram_tensor("consts", [K, 513], f32,
                                 kind="ExternalInput")
    out_dram = nc.dram_tensor("out", [K, 3], f32, kind="ExternalOutput")

    x_pkf = x_dram.ap().rearrange("k (p f) -> p k f", f=F)      # [128, K, 2048]
    x_rows = x_dram.ap().rearrange("k (r s) -> (k r) s", s=S)   # [K*512, 512]

    with tile.TileContext(nc) as tc, ExitStack() as ctx:
        const_pool = ctx.enter_context(tc.tile_pool(name="const", bufs=1))
        in_pool = ctx.enter_context(
            tc.tile_pool(name="in", bufs=len(TILES)))
        small_pool = ctx.enter_context(tc.tile_pool(name="small", bufs=1))
        psum_pool = ctx.enter_context(
            tc.tile_pool(name="psum", bufs=1, space="PSUM"))

        # stream DMAs first so descriptors hit the queues ASAP; one DMA per
        # keypoint (fast completion sems keep DVE close behind the stream),
        # the last keypoint in two half-DMAs so its reduce lags even less.
        tiles_sb = []
        for k in range(K):
            t = in_pool.tile([P, F], f32, tag="xin")
            if k < K - 1:
                nc.sync.dma_start(t[:], x_pkf[:, k, :])
            else:
                nc.sync.dma_start(t[:, 0:F // 2], x_pkf[:, k, 0:F // 2])
                nc.sync.dma_start(t[:, F // 2:F], x_pkf[:, k, F // 2:F])
            tiles_sb.append((k, t))

        ident = const_pool.tile([P, P], f32)
        nc.scalar.dma_start(ident[:], ident_dram.ap())
        consts = const_pool.tile([K, 513], f32)
        nc.scalar.dma_start(consts[:], consts_dram.ap())
        # riota_pc[k, c*128+p] = 512 - (4p + c): rank of chunk (p,c) in flat
        # order, reversed so the masked reduce_max picks the first occurrence.
        riota_pc = consts[:, 0:512]
        kiota = consts[:, 512:513]         # kiota[k] = 512*(k+1)

        out_sb = small_pool.tile([K, 3], f32)
        nc.vector.memset(out_sb[:, 0:2], -4.0)
        nc.vector.memset(out_sb[:, 2:3], -1.0)

        # per-(partition, chunk) maxes, chunk columns in keypoint-major order
        pmax = small_pool.tile([P, K * C], f32)
        psumT = psum_pool.tile([K, C * P], f32)
        pm3 = pmax[:].rearrange("p (k c) -> p k c", c=C)
        for k, t in tiles_sb:
            t3 = t[:].rearrange("p (c s) -> p c s", s=S)        # [P, C, S]
            if k < K - 1:
                nc.vector.reduce_max(
                    pmax[:, k * C:(k + 1) * C], t3[:, :, :], axis=X)
            else:
                nc.vector.reduce_max(
                    pmax[:, k * C:k * C + 2], t3[:, 0:2, :], axis=X)
                nc.vector.reduce_max(
                    pmax[:, k * C + 2:(k + 1) * C], t3[:, 2:C, :], axis=X)
        # transpose chunk columns -> psumT[k, c*128+p] = chunkmax(p, c)
        for c in range(C):
            nc.tensor.matmul(psumT[:, c * P:(c + 1) * P],
                             pm3[:, :, c], ident[:], is_transpose=True)

        # winning chunk per keypoint, in exact flat-rank tie order:
        # mask (>= gmax) * reversed-rank-iota, reduce_max -> 512 - rank
        gmax = small_pool.tile([K, 1], f32)
        nc.vector.reduce_max(gmax[:], psumT[:], axis=X)
        cand_p = small_pool.tile([K, C * P], f32)
        nc.vector.scalar_tensor_tensor(
            cand_p[:], in0=psumT[:], scalar=gmax[:], in1=riota_pc,
            op0=Alu.is_ge, op1=Alu.mult)
        rc = small_pool.tile([K, 1], f32)   # 512 - rank
        nc.vector.reduce_max(rc[:], cand_p[:], axis=X)
        # gather row = 512*k + rank = kiota - rc
        offs_i = small_pool.tile([K, 1], i32)
        nc.vector.scalar_tensor_tensor(
            offs_i[:], in0=rc[:], scalar=-1.0, in1=kiota,
            op0=Alu.mult, op1=Alu.add)

        # one gather for all 17 winning chunks
        grow = small_pool.tile([K, S], f32)
        nc.gpsimd.indirect_dma_start(
            out=grow[:], out_offset=None, in_=x_rows,
            in_offset=bass.IndirectOffsetOnAxis(ap=offs_i[:, 0:1], axis=0))

        # off-critical-path decode prep (runs while the gather is in flight)
        cand = small_pool.tile([K, 3], f32)
        nc.scalar.activation(cand[:, 2:3], gmax[:], Act.Sigmoid)
        # yy = rank (chunk size == W): 4*yy = 2048 - 4*rc
        nc.vector.tensor_scalar(cand[:, 1:2], rc[:], -4.0, 2048.0,
                                Alu.mult, Alu.add)
        valid = small_pool.tile([K, 1], f32)
        nc.vector.tensor_scalar(valid[:], cand[:, 2:3], 0.8, None, Alu.is_gt)
        vb3 = small_pool.tile([K, 3], i32)
        nc.vector.tensor_scalar(vb3[:], out_sb[:], 0.0, valid[:],
                                Alu.mult, Alu.add)
        nc.vector.copy_predicated(out_sb[:, 1:3], vb3[:, 1:3], cand[:, 1:3])

        # index within the winning chunk == xx
        jmax8 = small_pool.tile([K, 8], f32)
        nc.vector.max(jmax8[:], grow[:])
        jidx8 = small_pool.tile([K, 8], u32)
        nc.vector.max_index(jidx8[:], jmax8[:], grow[:])
        nc.vector.tensor_scalar(cand[:, 0:1], jidx8[:, 0:1], 4.0, None,
                                Alu.mult)
        nc.vector.copy_predicated(out_sb[:, 0:1], vb3[:, 0:1], cand[:, 0:1])
        nc.scalar.dma_start(out_dram.ap()[:, :], out_sb[:])

    nc.compile()
    return nc


def _get_nc():
    global _NC_CACHE
    if _NC_CACHE is None:
        _NC_CACHE = _build()
    return _NC_CACHE


def _shard(x: np.ndarray) -> list[dict[str, np.ndarray]]:
    xf = np.ascontiguousarray(np.asarray(x, dtype=np.float32).reshape(NK, ROW))
    ident = np.ascontiguousarray(np.eye(P, dtype=np.float32))
    riota_pc = np.zeros(512, np.float32)
    for c in range(C):
        for p in range(P):
            riota_pc[c * P + p] = S - (C * p + c)
    kiota = (float(S) * (np.arange(K) + 1)).astype(np.float32)
    consts = np.ascontiguousarray(np.concatenate(
        [np.tile(riota_pc, (K, 1)), kiota[:, None]], axis=1, dtype=np.float32))
    shards = []
    for c in range(N_CORES):
        lo = c * K
        s = xf[lo:min(lo + K, NK)]
        if s.shape[0] < K:
            s = np.concatenate(
                [s, np.zeros((K - s.shape[0], ROW), np.float32)], axis=0)
        shards.append({"x": np.ascontiguousarray(s),
                       "ident": ident, "consts": consts})
    return shards


def _run(x, trace=False, **kw):
    nc = _get_nc()
    res = run_bass_kernel_spmd(nc, _shard(x), core_ids=list(range(N_CORES)),
                               trace=trace, **kw)
    out = np.concatenate([r["out"] for r in res.results], axis=0)[:NK]
    return out.astype(np.float32), res


def kernel(x: np.ndarray) -> np.ndarray:
    out, _ = _run(x, trace=False)
    return out


# revision 15
# speedup vs baseline: 1.2264x; 1.0046x over previous
"""Trainium2 Bass kernel for nn_DecodeSBP (keypoint heatmap decode).

Contract: kernel(x=[1,133,512,512] f32) -> [133,3] f32
  joints[k] = (4*xx, 4*yy, conf) if conf > 0.8 else (-4, -4, -1)
  where flat = argmax(sigmoid(x[0,k])), conf = sigmoid(max), yy = flat//512,
  xx = flat%512. sigmoid is monotonic so the argmax runs on raw logits.

Sharding: keypoint dim across 8 cores (17/core, core 7 zero-padded).

Per-core program (one full-data pass, hierarchical argmax):
  stream: 7 tiles (3,3,3,3,3,1,1 keypoints). Per tile the per-(partition,
    512-chunk) max reduction is split: the Pool engine pre-halves most
    chunks with an elementwise tensor_max (512 -> 256), DVE reduce_max
    finishes those and fully reduces the rest, so the reduction keeps
    pace with the 17.8 MB DMA stream.
  finale (single group, once, after the last tile):
    TensorE-transpose the 4 chunk columns into one PSUM tile [17, 512]
    whose column c*128+p is chunkmax(p, c); DVE max/max_index pick the
    winning chunk per keypoint; integer ops turn the column into the
    chunk's rank 4p+c (= yy, since chunk size == image width); ONE
    indirect_dma_start gathers all 17 winning 2 KB chunks (row indices
    from SBUF) -- avoids per-keypoint register DMAs, each of which
    stalls all 16 DMA queues ~1.1 us; max/max_index on the gathered
    rows give xx; confidence-gated copy_predicated builds the output.
  The PE identity and the per-keypoint row base are host-provided
  inputs, removing the gpsimd iota preamble from the critical path.
"""

import sys
from contextlib import ExitStack

for _p in ("/opt/trn_rl_repo", "/opt/pypackages"):
    if _p not in sys.path:
        sys.path.append(_p)

import numpy as np

import concourse.bacc as bacc
import concourse.bass as bass
import concourse.tile as tile
from concourse import mybir
from concourse.bass_utils import run_bass_kernel_spmd

K = 17          # keypoints per core
NK = 133        # total keypoints
ROW = 262144    # 512*512
P = 128         # SBUF partitions
F = ROW // P    # 2048 free elems per partition
C = 4           # chunks per partition row
S = F // C     # 512 elems per chunk
W = 512
N_CORES = 8
TILES = (1,) * K   # one keypoint per stream tile

f32 = mybir.dt.float32
i32 = mybir.dt.int32
u32 = mybir.dt.uint32
Alu = mybir.AluOpType
Act = mybir.ActivationFunctionType
X = mybir.AxisListType.X

_NC_CACHE = None


def _build():
    nc = bacc.Bacc("TRN2", target_bir_lowering=False, debug=False)
    x_dram = nc.dram_tensor("x", [K, ROW], f32, kind="ExternalInput")
    ident_dram = nc.dram_tensor("ident", [P, P], f32, kind="ExternalInput")
    consts_dram = nc.dram_tensor("consts", [K, 513], f32,
                                 kind="ExternalInput")
    out_dram = nc.dram_tensor("out", [K, 3], f32, kind="ExternalOutput")

    x_pkf = x_dram.ap().rearrange("k (p f) -> p k f", f=F)      # [128, K, 2048]
    x_rows = x_dram.ap().rearrange("k (r s) -> (k r) s", s=S)   # [K*512, 512]

    with tile.TileContext(nc) as tc, ExitStack() as ctx:
        const_pool = ctx.enter_context(tc.tile_pool(name="const", bufs=1))
        in_pool = ctx.enter_context(
            tc.tile_pool(name="in", bufs=len(TILES)))
        small_pool = ctx.enter_context(tc.tile_pool(name="small", bufs=1))
        psum_pool = ctx.enter_context(
            tc.tile_pool(name="psum", bufs=1, space="PSUM"))

        # stream DMAs first so descriptors hit the queues ASAP; one DMA per
        # keypoint (fast completion sems keep DVE close behind the stream),
        # the last keypoint in two half-DMAs so its reduce lags even less.
        tiles_sb = []
        for k in range(K):
            t = in_pool.tile([P, F], f32, tag="xin")
            if k < K - 1:
                nc.sync.dma_start(t[:], x_pkf[:, k, :])
            else:
                for c in range(C):
                    nc.sync.dma_start(t[:, c * S:(c + 1) * S],
                                      x_pkf[:, k, c * S:(c + 1) * S])
            tiles_sb.append((k, t))

        ident = const_pool.tile([P, P], f32)
        nc.scalar.dma_start(ident[:], ident_dram.ap())
        consts = const_pool.tile([K, 513], f32)
        nc.scalar.dma_start(consts[:], consts_dram.ap())
        # riota_pc[k, c*128+p] = 512 - (4p + c): rank of chunk (p,c) in flat
        # order, reversed so the masked reduce_max picks the first occurrence.
        riota_pc = consts[:, 0:512]
        kiota = consts[:, 512:513]         # kiota[k] = 512*(k+1)

        out_sb = small_pool.tile([K, 3], f32)
        nc.vector.memset(out_sb[:, 0:2], -4.0)
        nc.vector.memset(out_sb[:, 2:3], -1.0)

        # per-(partition, chunk) maxes, chunk columns in keypoint-major order
        pmax = small_pool.tile([P, K * C], f32)
        psumT = psum_pool.tile([K, C * P], f32)
        pm3 = pmax[:].rearrange("p (k c) -> p k c", c=C)
        warm = psum_pool.tile([1, P], f32, tag="warm")
        for k, t in tiles_sb:
            t3 = t[:].rearrange("p (c s) -> p c s", s=S)        # [P, C, S]
            if k < K - 1:
                nc.vector.reduce_max(
                    pmax[:, k * C:(k + 1) * C], t3[:, :, :], axis=X)
            else:
                # quarter-reduces chase the four quarter-DMAs
                for c in range(C):
                    nc.vector.reduce_max(
                        pmax[:, k * C + c:k * C + c + 1],
                        t3[:, c:c + 1, :], axis=X)
            if k in (8, 13):
                # keep the PE p-state warm for the finale transposes
                nc.tensor.matmul(warm[:], pm3[:, 0:1, 0], ident[:],
                                 is_transpose=True)
        # transpose chunk columns -> psumT[k, c*128+p] = chunkmax(p, c)
        for c in range(C):
            nc.tensor.matmul(psumT[:, c * P:(c + 1) * P],
                             pm3[:, :, c], ident[:], is_transpose=True)

        # winning chunk per keypoint, in exact flat-rank tie order:
        # mask (>= gmax) * reversed-rank-iota, reduce_max -> 512 - rank
        gmax = small_pool.tile([K, 1], f32)
        nc.vector.reduce_max(gmax[:], psumT[:], axis=X)
        cand_p = small_pool.tile([K, C * P], f32)
        nc.vector.scalar_tensor_tensor(
            cand_p[:], in0=psumT[:], scalar=gmax[:], in1=riota_pc,
            op0=Alu.is_ge, op1=Alu.mult)
        rc = small_pool.tile([K, 1], f32)   # 512 - rank
        nc.vector.reduce_max(rc[:], cand_p[:], axis=X)
        # gather row = 512*k + rank = kiota - rc. The indirect-DMA offset AP
        # must start at its tile's partition 0, so the second gather's
        # offsets are stream-shuffled down to partitions 0..7.
        KH = 9
        offs32 = small_pool.tile([32, 1], i32)
        nc.vector.scalar_tensor_tensor(
            offs32[0:K, :], in0=rc[:], scalar=-1.0, in1=kiota,
            op0=Alu.mult, op1=Alu.add)
        offsb32 = small_pool.tile([32, 1], i32)
        nc.vector.stream_shuffle(
            offsb32[:], offs32[:],
            mask=list(range(KH, K)) + [0] * (32 - (K - KH)))

        # gather the 17 winning chunks; two DMAs so the 2 KB descriptors
        # drain through two queues instead of serializing on one
        grow = small_pool.tile([K, S], f32)
        nc.gpsimd.indirect_dma_start(
            out=grow[0:KH, :], out_offset=None, in_=x_rows,
            in_offset=bass.IndirectOffsetOnAxis(ap=offs32[0:KH, 0:1], axis=0))
        nc.gpsimd.indirect_dma_start(
            out=grow[KH:K, :], out_offset=None, in_=x_rows,
            in_offset=bass.IndirectOffsetOnAxis(
                ap=offsb32[0:K - KH, 0:1], axis=0))

        # off-critical-path decode prep (runs while the gather is in flight)
        cand = small_pool.tile([K, 3], f32)
        nc.scalar.activation(cand[:, 2:3], gmax[:], Act.Sigmoid)
        # yy = rank (chunk size == W): 4*yy = 2048 - 4*rc
        nc.vector.tensor_scalar(cand[:, 1:2], rc[:], -4.0, 2048.0,
                                Alu.mult, Alu.add)
        valid = small_pool.tile([K, 1], f32)
        nc.vector.tensor_scalar(valid[:], cand[:, 2:3], 0.8, None, Alu.is_gt)
        vb3 = small_pool.tile([K, 3], i32)
        nc.vector.tensor_scalar(vb3[:], out_sb[:], 0.0, valid[:],
                                Alu.mult, Alu.add)
        nc.vector.copy_predicated(out_sb[:, 1:3], vb3[:, 1:3], cand[:, 1:3])

        # index within the winning chunk == xx
        jmax8 = small_pool.tile([K, 8], f32)
        nc.vector.max(jmax8[:], grow[:])
        jidx8 = small_pool.tile([K, 8], u32)
        nc.vector.max_index(jidx8[:], jmax8[:], grow[:])
        nc.vector.tensor_scalar(cand[:, 0:1], jidx8[:, 0:1], 4.0, None,
                                Alu.mult)
        nc.vector.copy_predicated(out_sb[:, 0:1], vb3[:, 0:1], cand[:, 0:1])
        nc.scalar.dma_start(out_dram.ap()[:, :], out_sb[:])

    nc.compile()
    return nc


def _get_nc():
    global _NC_CACHE
    if _NC_CACHE is None:
        _NC_CACHE = _build()
    return _NC_CACHE


def _shard(x: np.ndarray) -> list[dict[str, np.ndarray]]:
    xf = np.ascontiguousarray(np.asarray(x, dtype=np.float32).reshape(NK, ROW))
    ident = np.ascontiguousarray(np.eye(P, dtype=np.float32))
    riota_pc = np.zeros(512, np.float32)
    for c in range(C):
        for p in range(P):
            riota_pc[c * P + p] = S - (C * p + c)
    kiota = (float(S) * (np.arange(K) + 1)).astype(np.float32)
    consts = np.ascontiguousarray(np.concatenate(
        [np.tile(riota_pc, (K, 1)), kiota[:, None]], axis=1, dtype=np.float32))
    shards = []
    for c in range(N_CORES):
        lo = c * K
        s = xf[lo:min(lo + K, NK)]
        if s.shape[0] < K:
            s = np.concatenate(
                [s, np.zeros((K - s.shape[0], ROW), np.float32)], axis=0)
        shards.append({"x": np.ascontiguousarray(s),
                       "ident": ident, "consts": consts})
    return shards


def _run(x, trace=False, **kw):
    nc = _get_nc()
    res = run_bass_kernel_spmd(nc, _shard(x), core_ids=list(range(N_CORES)),
                               trace=trace, **kw)
    out = np.concatenate([r["out"] for r in res.results], axis=0)[:NK]
    return out.astype(np.float32), res


def kernel(x: np.ndarray) -> np.ndarray:
    out, _ = _run(x, trace=False)
    return out
